# revision 1
# baseline (speedup 1.0000x reference)
"""MoE group-limited routing gate (DeepSeek-style) on 8 Trainium2 NeuronCores.

Computation (per token t over E=256 experts, D=7168 features):
    logits = x @ weight.T                      [T, E]
    group-limited top-k: 8 groups of 32 experts, keep top-4 groups by
    group-max, then top-8 experts among kept groups.
    weights = sigmoid(logits[sel]) normalized to sum 1, * 2.5
Returns (weights [T,8] f32, indices [T,8] int32) like the reference.

Strategy: data-parallel over tokens, 2048 tokens/core, gate weight
replicated.  x is pre-transposed on host to [D, T] so the contraction dim
lands on SBUF partitions.  Matmul precision options:
  - "fp16x3": x and w are split on host into fp16 (hi, lo*2^11) pairs;
    logits = hi@hi + 2^-11*(hi@lo2 + lo2@hi).  fp16 products are exact in
    the f32 PSUM accumulator, so the result carries ~f32-grade accuracy
    (~1e-6) at 3 bf16-rate passes, and index flips vs the f32 reference
    are ~zero.  DMA bytes are unchanged (2+2 B/elem).
  - "f32r": single-pass full-rate fp32 (13-bit-ish products) — fastest,
    but logit error ~2e-4 flips ~0.1% of top-k indices.
Top-k uses the DVE native max/max_index (top-8 sorted) instructions; the
group top-4 uses a threshold trick (4th-largest group-max) since sigmoid
is monotone and masking is additive on logits.
"""

import numpy as np
from contextlib import ExitStack

import concourse.bacc as bacc
import concourse.tile as tile
from concourse import mybir
from concourse.bass_utils import run_bass_kernel_spmd

N_CORES = 8
T_FULL = 16384
D = 7168
E = 256
G = 8            # expert groups
EPG = E // G     # experts per group = 32
TOPK = 8
TOPK_GROUPS = 4
ROUTE_SCALE = 2.5

P = 128
T = T_FULL // N_CORES       # 2048 tokens per core
KC = D // P                 # 56 contraction chunks
TB = 256                    # tokens per block
NB = T // TB                # 8 blocks
TPB = TB // P               # 2 token-tiles per block
KQ = 4                      # x DMA splits per block (finer-grained deps)
KCQ = KC // KQ              # 14 k-chunks per split
WQ = 8                      # weight DMA splits
WCQ = KC // WQ              # 7 k-chunks per split
NEG = -1.0e30
LO_SCALE = 2.0 ** 11        # host scales the fp16 lo term by this
PRECISION = "fp16x3"        # "fp16x3" | "f32r"

_CACHE = {}


def _emit_topk(nc, sc_pool, out_pool, scores, wout, iout, t0):
    """Group-limited top-k + normalize on a [128, 256] f32 logits tile."""
    f32 = mybir.dt.float32
    scores_g = scores.rearrange("p (g e) -> p g e", g=G)
    glog = sc_pool.tile([P, G], f32)
    nc.vector.reduce_max(out=glog, in_=scores_g, axis=mybir.AxisListType.X)
    gsort = sc_pool.tile([P, G], f32)
    nc.vector.max(out=gsort, in_=glog)
    # additive mask: 0 for kept groups (>= 4th-largest), -1e30 otherwise
    maskadd = sc_pool.tile([P, G], f32)
    nc.vector.tensor_scalar(
        out=maskadd,
        in0=glog,
        scalar1=gsort[:, TOPK_GROUPS - 1:TOPK_GROUPS],
        scalar2=NEG,
        op0=mybir.AluOpType.is_lt,
        op1=mybir.AluOpType.mult,
    )
    masked = sc_pool.tile([P, E], f32)
    nc.vector.tensor_add(
        masked.rearrange("p (g e) -> p g e", g=G),
        scores_g,
        maskadd.to_broadcast([P, G, EPG]),
    )
    top8 = sc_pool.tile([P, TOPK], f32)
    nc.vector.max(out=top8, in_=masked)
    idx = out_pool.tile([P, TOPK], mybir.dt.uint32)
    nc.vector.max_index(out=idx, in_max=top8, in_values=masked)
    sig = sc_pool.tile([P, TOPK], f32)
    nc.scalar.activation(
        out=sig, in_=top8, func=mybir.ActivationFunctionType.Sigmoid
    )
    ssum = sc_pool.tile([P, 1], f32)
    nc.vector.reduce_sum(out=ssum, in_=sig, axis=mybir.AxisListType.X)
    rec = sc_pool.tile([P, 1], f32)
    nc.vector.reciprocal(out=rec, in_=ssum)
    wres = out_pool.tile([P, TOPK], f32)
    nc.vector.tensor_scalar(
        out=wres,
        in0=sig,
        scalar1=rec[:, 0:1],
        scalar2=ROUTE_SCALE,
        op0=mybir.AluOpType.mult,
        op1=mybir.AluOpType.mult,
    )
    # outputs ride the SWDGE ring so the tiny writes never stall the
    # HWDGE ring that streams x
    nc.gpsimd.dma_start(out=wout[t0:t0 + P, :], in_=wres)
    nc.gpsimd.dma_start(out=iout[t0:t0 + P, :], in_=idx)


def _build_f32r():
    nc = bacc.Bacc("TRN2", target_bir_lowering=False, debug=False, num_devices=N_CORES)
    f32 = mybir.dt.float32
    f32r = mybir.dt.float32r
    xt = nc.dram_tensor("xt", [D, T], f32r, kind="ExternalInput").ap()
    wt = nc.dram_tensor("wt", [D, E], f32r, kind="ExternalInput").ap()
    wout = nc.dram_tensor("w_out", [T, TOPK], f32, kind="ExternalOutput").ap()
    iout = nc.dram_tensor("i_out", [T, TOPK], mybir.dt.uint32, kind="ExternalOutput").ap()

    xt_r = xt.rearrange("(k p) t -> p k t", p=P)
    wt_r = wt.rearrange("(k p) e -> p k e", p=P)

    with tile.TileContext(nc) as tc, ExitStack() as ctx:
        wt_pool = ctx.enter_context(tc.tile_pool(name="wt", bufs=1))
        xt_pool = ctx.enter_context(tc.tile_pool(name="xt", bufs=2))
        psum_pool = ctx.enter_context(tc.tile_pool(name="psum", bufs=4, space="PSUM"))
        sc_pool = ctx.enter_context(tc.tile_pool(name="scratch", bufs=3))
        out_pool = ctx.enter_context(tc.tile_pool(name="outs", bufs=4))

        wt_sb = []
        for q in range(WQ):
            wtile = wt_pool.tile([P, WCQ, E], f32r, tag=f"wt{q}")
            nc.sync.dma_start(out=wtile, in_=wt_r[:, q * WCQ:(q + 1) * WCQ, :])
            wt_sb.append(wtile)

        for b in range(NB):
            xq = []
            for q in range(KQ):
                xtile = xt_pool.tile([P, KCQ, TB], f32r, tag=f"xt{q}")
                nc.sync.dma_start(
                    out=xtile,
                    in_=xt_r[:, q * KCQ:(q + 1) * KCQ, b * TB:(b + 1) * TB],
                )
                xq.append(xtile)
            for j in range(TPB):
                psum = psum_pool.tile([P, E], f32)
                for k in range(KC):
                    lhsT = xq[k // KCQ][:, k % KCQ, j * P:(j + 1) * P]
                    rhs = wt_sb[k // WCQ][:, k % WCQ, :]
                    nc.tensor.matmul(psum, lhsT, rhs, start=(k == 0), stop=(k == KC - 1))
                _emit_topk(nc, sc_pool, out_pool, psum, wout, iout, b * TB + j * P)
    nc.compile()
    return nc


def _build_fp16x3():
    nc = bacc.Bacc("TRN2", target_bir_lowering=False, debug=False, num_devices=N_CORES)
    f32 = mybir.dt.float32
    f16 = mybir.dt.float16
    xh = nc.dram_tensor("xh", [D, T], f16, kind="ExternalInput").ap()
    xl = nc.dram_tensor("xl", [D, T], f16, kind="ExternalInput").ap()
    wh = nc.dram_tensor("wh", [D, E], f16, kind="ExternalInput").ap()
    wl = nc.dram_tensor("wl", [D, E], f16, kind="ExternalInput").ap()
    wout = nc.dram_tensor("w_out", [T, TOPK], f32, kind="ExternalOutput").ap()
    iout = nc.dram_tensor("i_out", [T, TOPK], mybir.dt.uint32, kind="ExternalOutput").ap()

    xh_r = xh.rearrange("(k p) t -> p k t", p=P)
    xl_r = xl.rearrange("(k p) t -> p k t", p=P)
    wh_r = wh.rearrange("(k p) e -> p k e", p=P)
    wl_r = wl.rearrange("(k p) e -> p k e", p=P)

    with tile.TileContext(nc) as tc, ExitStack() as ctx:
        wt_pool = ctx.enter_context(tc.tile_pool(name="wt", bufs=1))
        xt_pool = ctx.enter_context(tc.tile_pool(name="xt", bufs=2))
        # 4+4 slots = all 8 PSUM banks: block b's accumulators coexist with
        # block b-1's (whose xh@wl half is deferred one block, see below)
        psA_pool = ctx.enter_context(tc.tile_pool(name="psA", bufs=4, space="PSUM"))
        psB_pool = ctx.enter_context(tc.tile_pool(name="psB", bufs=4, space="PSUM"))
        sc_pool = ctx.enter_context(tc.tile_pool(name="scratch", bufs=3))
        out_pool = ctx.enter_context(tc.tile_pool(name="outs", bufs=4))

        # DMA emission order matters: the sync HWDGE ring drains FIFO, so
        # put the first weight quarter + block-0 x tiles up front to get the
        # PE computing within a few us, then stream the rest of the weights.
        def load_w(q, which):
            src, lst, tag = (
                (wh_r, wh_sb, f"wh{q}") if which == "h" else (wl_r, wl_sb, f"wl{q}")
            )
            wtile = wt_pool.tile([P, WCQ, E], f16, tag=tag)
            nc.sync.dma_start(out=wtile, in_=src[:, q * WCQ:(q + 1) * WCQ, :])
            lst.append(wtile)

        def load_x_block(b):
            xh_q, xl_q = [], []
            t_lo, t_hi = b * TB, (b + 1) * TB
            for q in range(KQ):
                xtile = xt_pool.tile([P, KCQ, TB], f16, tag=f"xh{q}")
                nc.sync.dma_start(
                    out=xtile, in_=xh_r[:, q * KCQ:(q + 1) * KCQ, t_lo:t_hi]
                )
                xh_q.append(xtile)
                ltile = xt_pool.tile([P, KCQ, TB], f16, tag=f"xl{q}")
                nc.sync.dma_start(
                    out=ltile, in_=xl_r[:, q * KCQ:(q + 1) * KCQ, t_lo:t_hi]
                )
                xl_q.append(ltile)
            return xh_q, xl_q

        # emission order == consumption order: (wh_q, xh_q) pairs feed pass A,
        # then xl (pass B first half reuses resident wh), then wl quarters.
        wh_sb, wl_sb = [], []
        xh0, xl0 = [], []
        t_hi0 = TB
        for q in range(KQ):
            load_w(2 * q, "h")
            load_w(2 * q + 1, "h")
            xtile = xt_pool.tile([P, KCQ, TB], f16, tag=f"xh{q}")
            nc.sync.dma_start(out=xtile, in_=xh_r[:, q * KCQ:(q + 1) * KCQ, 0:t_hi0])
            xh0.append(xtile)
        for q in range(KQ):
            ltile = xt_pool.tile([P, KCQ, TB], f16, tag=f"xl{q}")
            nc.sync.dma_start(out=ltile, in_=xl_r[:, q * KCQ:(q + 1) * KCQ, 0:t_hi0])
            xl0.append(ltile)
        for q in range(WQ):
            load_w(q, "l")
        blocks = {0: (xh0, xl0)}

        # Per block: pass A (xh@wh) and pass B first half (xl@wh) run with
        # only the early-arriving wh stream; the xh@wl half is deferred one
        # block so nothing on the critical path waits for the wl quarters.
        def flush(state):
            bb, xh_q, psA_list, psB_list = state
            for j in range(TPB):
                js = slice(j * P, (j + 1) * P)
                psumB = psB_list[j]
                for k in range(KC):
                    nc.tensor.matmul(
                        psumB,
                        xh_q[k // KCQ][:, k % KCQ, js],
                        wl_sb[k // WCQ][:, k % WCQ, :],
                        start=False,
                        stop=(k == KC - 1),
                    )
                scores = sc_pool.tile([P, E], f32)
                nc.scalar.activation(
                    out=scores,
                    in_=psumB,
                    func=mybir.ActivationFunctionType.Copy,
                    scale=1.0 / LO_SCALE,
                )
                nc.vector.tensor_add(scores, scores, psA_list[j])
                _emit_topk(nc, sc_pool, out_pool, scores, wout, iout, bb * TB + j * P)

        # Block 0 runs A + B1 only (no wl dependency) and its xh@wl half is
        # flushed right after block 1's A pass, by which time the wl stream
        # has landed.  Blocks >= 1 use the tight per-tile A,B1+B2 structure
        # so tile release (and thus the next block's DMA prefetch) stays a
        # full block ahead.
        pending = None
        for b in range(NB):
            if b not in blocks:
                blocks[b] = load_x_block(b)
            xh_q, xl_q = blocks.pop(b)
            if b == 0:
                psA_list, psB_list = [], []
                for j in range(TPB):
                    js = slice(j * P, (j + 1) * P)
                    psumA = psA_pool.tile([P, E], f32)
                    for k in range(KC):
                        nc.tensor.matmul(
                            psumA,
                            xh_q[k // KCQ][:, k % KCQ, js],
                            wh_sb[k // WCQ][:, k % WCQ, :],
                            start=(k == 0),
                            stop=(k == KC - 1),
                        )
                    psA_list.append(psumA)
                for j in range(TPB):
                    js = slice(j * P, (j + 1) * P)
                    psumB = psB_pool.tile([P, E], f32)
                    for k in range(KC):
                        nc.tensor.matmul(
                            psumB,
                            xl_q[k // KCQ][:, k % KCQ, js],
                            wh_sb[k // WCQ][:, k % WCQ, :],
                            start=(k == 0),
                            stop=False,
                        )
                    psB_list.append(psumB)
                pending = (b, xh_q, psA_list, psB_list)
                continue
            for j in range(TPB):
                js = slice(j * P, (j + 1) * P)
                psumA = psA_pool.tile([P, E], f32)
                for k in range(KC):
                    nc.tensor.matmul(
                        psumA,
                        xh_q[k // KCQ][:, k % KCQ, js],
                        wh_sb[k // WCQ][:, k % WCQ, :],
                        start=(k == 0),
                        stop=(k == KC - 1),
                    )
                if pending is not None:
                    flush(pending)
                    pending = None
                psumB = psB_pool.tile([P, E], f32)
                for i in range(2 * KC):
                    k = i % KC
                    if i < KC:
                        lhsT = xl_q[k // KCQ][:, k % KCQ, js]
                        rhs = wh_sb[k // WCQ][:, k % WCQ, :]
                    else:
                        lhsT = xh_q[k // KCQ][:, k % KCQ, js]
                        rhs = wl_sb[k // WCQ][:, k % WCQ, :]
                    nc.tensor.matmul(
                        psumB, lhsT, rhs, start=(i == 0), stop=(i == 2 * KC - 1)
                    )
                scores = sc_pool.tile([P, E], f32)
                nc.scalar.activation(
                    out=scores,
                    in_=psumB,
                    func=mybir.ActivationFunctionType.Copy,
                    scale=1.0 / LO_SCALE,
                )
                nc.vector.tensor_add(scores, scores, psumA)
                _emit_topk(nc, sc_pool, out_pool, scores, wout, iout, b * TB + j * P)
    nc.compile()
    return nc


def _get_program(precision):
    key = f"nc_{precision}"
    if key not in _CACHE:
        _CACHE[key] = _build_fp16x3() if precision == "fp16x3" else _build_f32r()
    return _CACHE[key]


def _split_f16(a):
    hi = a.astype(np.float16)
    lo = ((a - hi.astype(np.float32)) * np.float32(LO_SCALE)).astype(np.float16)
    return hi, lo


def kernel(x: np.ndarray, weight: np.ndarray, _trace: bool = False, **_kw):
    x = np.asarray(x, dtype=np.float32)
    weight = np.asarray(weight, dtype=np.float32)
    assert x.shape == (T_FULL, D) and weight.shape == (E, D)

    nc = _get_program(PRECISION)
    xt_full = np.ascontiguousarray(x.T)              # [D, T_FULL]
    wt_host = np.ascontiguousarray(weight.T)         # [D, E]
    if PRECISION == "fp16x3":
        xh_full, xl_full = _split_f16(xt_full)
        wh_host, wl_host = _split_f16(wt_host)
        in_maps = [
            {
                "xh": np.ascontiguousarray(xh_full[:, c * T:(c + 1) * T]),
                "xl": np.ascontiguousarray(xl_full[:, c * T:(c + 1) * T]),
                "wh": wh_host,
                "wl": wl_host,
            }
            for c in range(N_CORES)
        ]
    else:
        in_maps = [
            {
                "xt": np.ascontiguousarray(xt_full[:, c * T:(c + 1) * T]),
                "wt": wt_host,
            }
            for c in range(N_CORES)
        ]
    if _trace:
        import prof

        results, exec_time_ns, percore, neff_dir = prof.profiled_run(
            nc, in_maps, core_ids=list(range(N_CORES))
        )
        _CACHE["last_result"] = {
            "exec_time_ns": exec_time_ns,
            "percore": percore,
            "neff_dir": neff_dir,
        }
    else:
        res = run_bass_kernel_spmd(nc, in_maps, core_ids=list(range(N_CORES)))
        results = res.results
    w_full = np.concatenate([results[c]["w_out"] for c in range(N_CORES)], axis=0)
    i_full = np.concatenate(
        [results[c]["i_out"].astype(np.int32) for c in range(N_CORES)], axis=0
    )
    return w_full, i_full



# revision 3
# speedup vs baseline: 1.3549x; 1.3549x over previous
"""MoE group-limited routing gate (DeepSeek-style) on 8 Trainium2 NeuronCores.

Computation (per token t over E=256 experts, D=7168 features):
    logits = x @ weight.T                      [T, E]
    group-limited top-k: 8 groups of 32 experts, keep top-4 groups by
    group-max, then top-8 experts among kept groups.
    weights = sigmoid(logits[sel]) normalized to sum 1, * 2.5
Returns (weights [T,8] f32, indices [T,8] int32) like the reference.

Strategy: data-parallel over tokens, 2048 tokens/core, gate weight
replicated.  Matmul runs as one fp16 pass plus two fp8-e4m3 passes in
DoubleRow mode (2 k-chunks per instruction at 0.5 cycles/row), all
accumulating into a single PSUM tile via power-of-2 scale alignment:

    psum = xh @ (w<<16)_fp16  +  (xl<<11)_fp8 @ (w<<5)_fp8
                              +  (xh)_fp8 @ ((w - wh)<<16)_fp8
    logits = psum * 2^-16

fp16/fp8 products are exact in the f32 PSUM accumulator; the remaining
error is the fp8 rounding of the correction terms (~2e-5 absolute on
logits whose std is ~2.0), flipping ~7 of 131072 top-k slots.

Host pre-transposes into [128, block, chunk, token] layout so every DMA
lands 3.5-7KB contiguous runs per partition (vs 256-512B for a naive
[D, T] layout).  Top-k runs directly on the scaled PSUM (selection is
scale-invariant); the 2^-16 unscale rides the sigmoid activation's
scale input for free.
"""

import os
import numpy as np
from contextlib import ExitStack

import ml_dtypes

import concourse.bacc as bacc
import concourse.tile as tile
from concourse import mybir
from concourse.bass_utils import run_bass_kernel_spmd

N_CORES = 8
T_FULL = 16384
D = 7168
E = 256
G = 8            # expert groups
EPG = E // G     # experts per group = 32
TOPK = 8
TOPK_GROUPS = 4
ROUTE_SCALE = 2.5

P = 128
T = T_FULL // N_CORES       # 2048 tokens per core
KC = D // P                 # 56 contraction chunks
TB = 256                    # tokens per block
NB = T // TB                # 8 blocks per core
NBF = T_FULL // TB          # 64 blocks total
TPB = TB // P               # 2 token-tiles per block
KQ = 4                      # DMA splits per block / weight tensor
KCQ = KC // KQ              # 14 k-chunks per split (even: DoubleRow pairs)
NEG = -1.0e30
S16 = 2.0 ** 16             # scale of the PSUM accumulator
XL_SCALE = 2.0 ** 11        # xl fp8 pre-scale
W8_SCALE = 2.0 ** 5         # w8 fp8 pre-scale  (XL_SCALE*W8_SCALE == S16)
F8NP = ml_dtypes.float8_e4m3

PRECISION = os.environ.get("KPREC", "hybrid8")

_CACHE = {}


def _emit_topk(nc, sc_pool, out_pool, scores, wout, iout, t0, scale=1.0):
    """Group-limited top-k + normalize on a [128, 256] f32 logits tile.

    scores may hold logits * (1/scale); selection is scale-invariant and
    the unscale is folded into the sigmoid activation."""
    f32 = mybir.dt.float32
    scores_g = scores.rearrange("p (g e) -> p g e", g=G)
    glog = sc_pool.tile([P, G], f32)
    nc.vector.reduce_max(out=glog, in_=scores_g, axis=mybir.AxisListType.X)
    gsort = sc_pool.tile([P, G], f32)
    nc.vector.max(out=gsort, in_=glog)
    # additive mask: 0 for kept groups (>= 4th-largest), -1e30 otherwise
    maskadd = sc_pool.tile([P, G], f32)
    nc.vector.tensor_scalar(
        out=maskadd,
        in0=glog,
        scalar1=gsort[:, TOPK_GROUPS - 1:TOPK_GROUPS],
        scalar2=NEG,
        op0=mybir.AluOpType.is_lt,
        op1=mybir.AluOpType.mult,
    )
    masked = sc_pool.tile([P, E], f32)
    nc.vector.tensor_add(
        masked.rearrange("p (g e) -> p g e", g=G),
        scores_g,
        maskadd.to_broadcast([P, G, EPG]),
    )
    top8 = sc_pool.tile([P, TOPK], f32)
    nc.vector.max(out=top8, in_=masked)
    idx = out_pool.tile([P, TOPK], mybir.dt.uint32)
    nc.vector.max_index(out=idx, in_max=top8, in_values=masked)
    sig = sc_pool.tile([P, TOPK], f32)
    nc.scalar.activation(
        out=sig, in_=top8, func=mybir.ActivationFunctionType.Sigmoid, scale=scale
    )
    ssum = sc_pool.tile([P, 1], f32)
    nc.vector.reduce_sum(out=ssum, in_=sig, axis=mybir.AxisListType.X)
    rec = sc_pool.tile([P, 1], f32)
    nc.vector.reciprocal(out=rec, in_=ssum)
    wres = out_pool.tile([P, TOPK], f32)
    nc.vector.tensor_scalar(
        out=wres,
        in0=sig,
        scalar1=rec[:, 0:1],
        scalar2=ROUTE_SCALE,
        op0=mybir.AluOpType.mult,
        op1=mybir.AluOpType.mult,
    )
    # outputs ride the SWDGE ring so the tiny writes never stall the
    # HWDGE ring that streams x
    nc.gpsimd.dma_start(out=wout[t0:t0 + P, :], in_=wres)
    nc.gpsimd.dma_start(out=iout[t0:t0 + P, :], in_=idx)


def _build_hybrid8():
    nc = bacc.Bacc("TRN2", target_bir_lowering=False, debug=False, num_devices=N_CORES)
    f32 = mybir.dt.float32
    f16 = mybir.dt.float16
    f8 = mybir.dt.float8e4
    DR = mybir.MatmulPerfMode.DoubleRow

    xh = nc.dram_tensor("xh", [P, NB, KC, TB], f16, kind="ExternalInput").ap()
    xl = nc.dram_tensor("xl", [P, NB, KC, TB], f8, kind="ExternalInput").ap()
    x8 = nc.dram_tensor("x8", [P, NB, KC, TB], f8, kind="ExternalInput").ap()
    wh = nc.dram_tensor("wh", [P, KC, E], f16, kind="ExternalInput").ap()
    w8 = nc.dram_tensor("w8", [P, KC, E], f8, kind="ExternalInput").ap()
    wl = nc.dram_tensor("wl", [P, KC, E], f8, kind="ExternalInput").ap()
    wout = nc.dram_tensor("w_out", [T, TOPK], f32, kind="ExternalOutput").ap()
    iout = nc.dram_tensor("i_out", [T, TOPK], mybir.dt.uint32, kind="ExternalOutput").ap()

    with tile.TileContext(nc) as tc, ExitStack() as ctx:
        wt_pool = ctx.enter_context(tc.tile_pool(name="wt", bufs=1))
        xh_pool = ctx.enter_context(tc.tile_pool(name="xh", bufs=2))
        xc_pool = ctx.enter_context(tc.tile_pool(name="xc", bufs=2))
        psum_pool = ctx.enter_context(tc.tile_pool(name="psum", bufs=6, space="PSUM"))
        sc_pool = ctx.enter_context(tc.tile_pool(name="scratch", bufs=3))
        out_pool = ctx.enter_context(tc.tile_pool(name="outs", bufs=4))

        wh_sb, w8_sb, wl_sb = [], [], []

        def load_w(lst, src, dt_, name, q):
            t = wt_pool.tile([P, KCQ, E], dt_, tag=f"{name}{q}")
            nc.sync.dma_start(out=t, in_=src[:, q * KCQ:(q + 1) * KCQ, :])
            lst.append(t)

        xh_blk, xl_blk, x8_blk = {}, {}, {}

        def load_x_split(dst, pool, src, dt_, name, b, q):
            t = pool.tile([P, KCQ, TB], dt_, tag=f"{name}{q}")
            nc.sync.dma_start(out=t, in_=src[:, b, q * KCQ:(q + 1) * KCQ, :])
            dst.setdefault(b, []).append(t)

        def load_x(dst, pool, src, dt_, name, b):
            for q in range(KQ):
                load_x_split(dst, pool, src, dt_, name, b, q)

        # DMA emission order == HWDGE consumption order: wh+xh(0)
        # interleaved (PE can start P1(0) within ~5us), xh(1), then the
        # correction streams for block 0, then steady-state per-block.
        for q in range(KQ):
            load_w(wh_sb, wh, f16, "wh", q)
            load_x_split(xh_blk, xh_pool, xh, f16, "xh", 0, q)
        load_x(xh_blk, xh_pool, xh, f16, "xh", 1)
        for q in range(KQ):
            load_w(w8_sb, w8, f8, "w8", q)
        load_x(xl_blk, xc_pool, xl, f8, "xl", 0)
        for q in range(KQ):
            load_w(wl_sb, wl, f8, "wl", q)
        load_x(x8_blk, xc_pool, x8, f8, "x8", 0)

        psums = {}

        def p1(b):
            ps_list = []
            for j in range(TPB):
                js = slice(j * P, (j + 1) * P)
                ps = psum_pool.tile([P, E], f32)
                for k in range(KC):
                    nc.tensor.matmul(
                        ps,
                        xh_blk[b][k // KCQ][:, k % KCQ, js],
                        wh_sb[k // KCQ][:, k % KCQ, :],
                        start=(k == 0),
                        stop=False,
                    )
                ps_list.append(ps)
            psums[b] = ps_list

        def corr(b):
            for j in range(TPB):
                js = slice(j * P, (j + 1) * P)
                ps = psums[b][j]
                for q in range(KC // 2):
                    s = 2 * q
                    sp, so = s // KCQ, s % KCQ
                    nc.tensor.matmul(
                        ps,
                        xl_blk[b][sp][:, so:so + 2, js],
                        w8_sb[sp][:, so:so + 2, :],
                        start=False,
                        stop=False,
                        perf_mode=DR,
                    )
                for q in range(KC // 2):
                    s = 2 * q
                    sp, so = s // KCQ, s % KCQ
                    nc.tensor.matmul(
                        ps,
                        x8_blk[b][sp][:, so:so + 2, js],
                        wl_sb[sp][:, so:so + 2, :],
                        start=False,
                        stop=(q == KC // 2 - 1),
                        perf_mode=DR,
                    )
                _emit_topk(
                    nc, sc_pool, out_pool, ps, wout, iout, b * TB + j * P,
                    scale=1.0 / S16,
                )

        # software pipeline: P1(b+1) runs while block b's correction
        # streams land; corrections of b run while xh(b+2) lands.
        p1(0)
        for b in range(NB):
            if b + 1 < NB:
                if b + 2 < NB:
                    load_x(xh_blk, xh_pool, xh, f16, "xh", b + 2)
                load_x(xl_blk, xc_pool, xl, f8, "xl", b + 1)
                load_x(x8_blk, xc_pool, x8, f8, "x8", b + 1)
                p1(b + 1)
            corr(b)
    nc.compile()
    return nc


def _get_program(precision):
    key = f"nc_{precision}"
    if key not in _CACHE:
        _CACHE[key] = _build_hybrid8()
    return _CACHE[key]


def _xlayout(a, c):
    """[D, T_FULL] -> per-core [P, NB, KC, TB] (d = k*P + p, t = b*TB + tt)."""
    return np.ascontiguousarray(
        a.reshape(KC, P, NBF, TB)[:, :, c * NB:(c + 1) * NB, :].transpose(1, 2, 0, 3)
    )


def _wlayout(a):
    """[D, E] -> [P, KC, E]."""
    return np.ascontiguousarray(a.reshape(KC, P, E).transpose(1, 0, 2))


def kernel(x: np.ndarray, weight: np.ndarray, _trace: bool = False, **_kw):
    x = np.asarray(x, dtype=np.float32)
    weight = np.asarray(weight, dtype=np.float32)
    assert x.shape == (T_FULL, D) and weight.shape == (E, D)

    nc = _get_program(PRECISION)

    xt = np.ascontiguousarray(x.T)                       # [D, T_FULL]
    xh_full = xt.astype(np.float16)
    xl_full = ((xt - xh_full.astype(np.float32)) * np.float32(XL_SCALE)).astype(F8NP)
    x8_full = xh_full.astype(F8NP)

    wt = np.ascontiguousarray(weight.T)                  # [D, E]
    wt_s = wt * np.float32(S16)
    wh_flat = wt_s.astype(np.float16)
    wl_flat = (wt_s - wh_flat.astype(np.float32)).astype(F8NP)
    w8_flat = (wt * np.float32(W8_SCALE)).astype(F8NP)
    wh_h = _wlayout(wh_flat)
    w8_h = _wlayout(w8_flat)
    wl_h = _wlayout(wl_flat)

    in_maps = [
        {
            "xh": _xlayout(xh_full, c),
            "xl": _xlayout(xl_full, c),
            "x8": _xlayout(x8_full, c),
            "wh": wh_h,
            "w8": w8_h,
            "wl": wl_h,
        }
        for c in range(N_CORES)
    ]
    if _trace:
        import prof

        results, exec_time_ns, percore, neff_dir = prof.profiled_run(
            nc, in_maps, core_ids=list(range(N_CORES))
        )
        _CACHE["last_result"] = {
            "exec_time_ns": exec_time_ns,
            "percore": percore,
            "neff_dir": neff_dir,
        }
    else:
        res = run_bass_kernel_spmd(nc, in_maps, core_ids=list(range(N_CORES)))
        results = res.results
    w_full = np.concatenate([results[c]["w_out"] for c in range(N_CORES)], axis=0)
    i_full = np.concatenate(
        [results[c]["i_out"].astype(np.int32) for c in range(N_CORES)], axis=0
    )
    return w_full, i_full


# revision 7
# speedup vs baseline: 1.5640x; 1.1544x over previous
"""MoE group-limited routing gate (DeepSeek-style) on 8 Trainium2 NeuronCores.

Computation (per token t over E=256 experts, D=7168 features):
    logits = x @ weight.T                      [T, E]
    group-limited top-k: 8 groups of 32 experts, keep top-4 groups by
    group-max, then top-8 experts among kept groups.
    weights = sigmoid(logits[sel]) normalized to sum 1, * 2.5
Returns (weights [T,8] f32, indices [T,8] int32) like the reference.

Strategy: data-parallel over tokens, 2048 tokens/core, gate weight
replicated.  Matmul runs as one fp16 pass plus two fp8-e4m3 passes in
DoubleRow mode (2 k-chunks per instruction at 0.5 cycles/row), all
accumulating into a single PSUM tile via power-of-2 scale alignment:

    psum = xh @ (w<<16)_fp16  +  (xl<<11)_fp8 @ (w<<5)_fp8
                              +  (xh)_fp8 @ ((w - wh)<<16)_fp8
    logits = psum * 2^-16

fp16/fp8 products are exact in the f32 PSUM accumulator; the remaining
error is the fp8 rounding of the correction terms (~2e-5 absolute on
logits whose std is ~2.0), flipping ~7 of 131072 top-k slots.

Host pre-transposes into [128, block, chunk, token] layout so every DMA
lands 3.5-7KB contiguous runs per partition (vs 256-512B for a naive
[D, T] layout).  Top-k runs directly on the scaled PSUM (selection is
scale-invariant); the 2^-16 unscale rides the sigmoid activation's
scale input for free.
"""

import os
import numpy as np
from contextlib import ExitStack

import ml_dtypes

import concourse.bacc as bacc
import concourse.tile as tile
from concourse import mybir
from concourse.bass_utils import run_bass_kernel_spmd

N_CORES = 8
T_FULL = 16384
D = 7168
E = 256
G = 8            # expert groups
EPG = E // G     # experts per group = 32
TOPK = 8
TOPK_GROUPS = 4
ROUTE_SCALE = 2.5

P = 128
T = T_FULL // N_CORES       # 2048 tokens per core
KC = D // P                 # 56 contraction chunks
TB = 256                    # tokens per block
NB = T // TB                # 8 blocks per core
NBF = T_FULL // TB          # 64 blocks total
TPB = TB // P               # 2 token-tiles per block
KQ = 7                      # DMA splits per block / weight tensor
KCQ = KC // KQ              # 8 k-chunks per split (even: DoubleRow pairs)
NEG = -1.0e30
S16 = 2.0 ** 16             # scale of the PSUM accumulator
XL_SCALE = 2.0 ** 11        # xl fp8 pre-scale
W8_SCALE = 2.0 ** 5         # w8 fp8 pre-scale  (XL_SCALE*W8_SCALE == S16)
F8NP = ml_dtypes.float8_e4m3

PRECISION = os.environ.get("KPREC", "hybrid8")

_CACHE = {}


def _emit_topk(nc, sc_pool, out_pool, scores, wout, iout, t0, scale=1.0,
               out_eng=None):
    """Group-limited top-k + normalize on a [128, 256] f32 logits tile.

    scores may hold logits * (1/scale); selection is scale-invariant and
    the unscale is folded into the sigmoid activation."""
    f32 = mybir.dt.float32
    scores_g = scores.rearrange("p (g e) -> p g e", g=G)
    glog = sc_pool.tile([P, G], f32)
    nc.vector.reduce_max(out=glog, in_=scores_g, axis=mybir.AxisListType.X)
    gsort = sc_pool.tile([P, G], f32)
    nc.vector.max(out=gsort, in_=glog)
    # additive mask: 0 for kept groups (>= 4th-largest), -1e30 otherwise
    maskadd = sc_pool.tile([P, G], f32)
    nc.vector.tensor_scalar(
        out=maskadd,
        in0=glog,
        scalar1=gsort[:, TOPK_GROUPS - 1:TOPK_GROUPS],
        scalar2=NEG,
        op0=mybir.AluOpType.is_lt,
        op1=mybir.AluOpType.mult,
    )
    masked = sc_pool.tile([P, E], f32)
    nc.vector.tensor_add(
        masked.rearrange("p (g e) -> p g e", g=G),
        scores_g,
        maskadd.to_broadcast([P, G, EPG]),
    )
    top8 = sc_pool.tile([P, TOPK], f32)
    nc.vector.max(out=top8, in_=masked)
    idx = out_pool.tile([P, TOPK], mybir.dt.uint32)
    nc.vector.max_index(out=idx, in_max=top8, in_values=masked)
    sig = sc_pool.tile([P, TOPK], f32)
    nc.scalar.activation(
        out=sig, in_=top8, func=mybir.ActivationFunctionType.Sigmoid, scale=scale
    )
    ssum = sc_pool.tile([P, 1], f32)
    nc.vector.reduce_sum(out=ssum, in_=sig, axis=mybir.AxisListType.X)
    rec = sc_pool.tile([P, 1], f32)
    nc.vector.reciprocal(out=rec, in_=ssum)
    wres = out_pool.tile([P, TOPK], f32)
    nc.vector.tensor_scalar(
        out=wres,
        in0=sig,
        scalar1=rec[:, 0:1],
        scalar2=ROUTE_SCALE,
        op0=mybir.AluOpType.mult,
        op1=mybir.AluOpType.mult,
    )
    # outputs ride the SWDGE ring so the tiny writes never stall the
    # HWDGE ring that streams x; the last block uses the (by then idle)
    # sync HWDGE ring instead so the SWDGE drain overlaps compute.
    eng = out_eng if out_eng is not None else nc.gpsimd
    eng.dma_start(out=wout[t0:t0 + P, :], in_=wres)
    eng.dma_start(out=iout[t0:t0 + P, :], in_=idx)


def _build_hybrid8():
    nc = bacc.Bacc("TRN2", target_bir_lowering=False, debug=False, num_devices=N_CORES)
    f32 = mybir.dt.float32
    f16 = mybir.dt.float16
    f8 = mybir.dt.float8e4
    DR = mybir.MatmulPerfMode.DoubleRow

    xh = nc.dram_tensor("xh", [P, NB, KC, TB], f16, kind="ExternalInput").ap()
    xl = nc.dram_tensor("xl", [P, NB, KC, TB], f8, kind="ExternalInput").ap()
    x8 = nc.dram_tensor("x8", [P, NB, KC, TB], f8, kind="ExternalInput").ap()
    wh = nc.dram_tensor("wh", [P, KC, E], f16, kind="ExternalInput").ap()
    w8 = nc.dram_tensor("w8", [P, KC, E], f8, kind="ExternalInput").ap()
    wl = nc.dram_tensor("wl", [P, KC, E], f8, kind="ExternalInput").ap()
    wout = nc.dram_tensor("w_out", [T, TOPK], f32, kind="ExternalOutput").ap()
    iout = nc.dram_tensor("i_out", [T, TOPK], mybir.dt.uint32, kind="ExternalOutput").ap()

    with tile.TileContext(nc) as tc, ExitStack() as ctx:
        wt_pool = ctx.enter_context(tc.tile_pool(name="wt", bufs=1))
        xh_pool = ctx.enter_context(tc.tile_pool(name="xh", bufs=2))
        xc_pool = ctx.enter_context(tc.tile_pool(name="xc", bufs=2))
        psum_pool = ctx.enter_context(tc.tile_pool(name="psum", bufs=6, space="PSUM"))
        sc_pool = ctx.enter_context(tc.tile_pool(name="scratch", bufs=3))
        out_pool = ctx.enter_context(tc.tile_pool(name="outs", bufs=4))

        wh_sb, w8_sb, wl_sb = [], [], []

        def load_w(lst, src, dt_, name, q):
            t = wt_pool.tile([P, KCQ, E], dt_, tag=f"{name}{q}")
            nc.sync.dma_start(out=t, in_=src[:, q * KCQ:(q + 1) * KCQ, :])
            lst.append(t)

        xh_blk, xl_blk, x8_blk = {}, {}, {}

        def load_x_split(dst, pool, src, dt_, name, b, q):
            t = pool.tile([P, KCQ, TB], dt_, tag=f"{name}{q}")
            nc.sync.dma_start(out=t, in_=src[:, b, q * KCQ:(q + 1) * KCQ, :])
            dst.setdefault(b, []).append(t)

        def load_x(dst, pool, src, dt_, name, b):
            for q in range(KQ):
                load_x_split(dst, pool, src, dt_, name, b, q)

        # DMA emission order == HWDGE consumption order: wh+xh(0)
        # interleaved (PE can start P1(0) within ~5us), xh(1), then the
        # correction streams for block 0, then steady-state per-block.
        for q in range(KQ):
            load_w(wh_sb, wh, f16, "wh", q)
            load_x_split(xh_blk, xh_pool, xh, f16, "xh", 0, q)
        load_x(xh_blk, xh_pool, xh, f16, "xh", 1)
        for q in range(KQ):
            load_w(w8_sb, w8, f8, "w8", q)
        load_x(xl_blk, xc_pool, xl, f8, "xl", 0)
        for q in range(KQ):
            load_w(wl_sb, wl, f8, "wl", q)
        load_x(x8_blk, xc_pool, x8, f8, "x8", 0)

        psums = {}

        def p1(b):
            ps_list = []
            for j in range(TPB):
                js = slice(j * P, (j + 1) * P)
                ps = psum_pool.tile([P, E], f32)
                for k in range(KC):
                    nc.tensor.matmul(
                        ps,
                        xh_blk[b][k // KCQ][:, k % KCQ, js],
                        wh_sb[k // KCQ][:, k % KCQ, :],
                        start=(k == 0),
                        stop=False,
                    )
                ps_list.append(ps)
            psums[b] = ps_list

        def corr(b):
            for j in range(TPB):
                js = slice(j * P, (j + 1) * P)
                ps = psums[b][j]
                for q in range(KC // 2):
                    s = 2 * q
                    sp, so = s // KCQ, s % KCQ
                    nc.tensor.matmul(
                        ps,
                        xl_blk[b][sp][:, so:so + 2, js],
                        w8_sb[sp][:, so:so + 2, :],
                        start=False,
                        stop=False,
                        perf_mode=DR,
                    )
                for q in range(KC // 2):
                    s = 2 * q
                    sp, so = s // KCQ, s % KCQ
                    nc.tensor.matmul(
                        ps,
                        x8_blk[b][sp][:, so:so + 2, js],
                        wl_sb[sp][:, so:so + 2, :],
                        start=False,
                        stop=(q == KC // 2 - 1),
                        perf_mode=DR,
                    )
                _emit_topk(
                    nc, sc_pool, out_pool, ps, wout, iout, b * TB + j * P,
                    scale=1.0 / S16,
                    out_eng=nc.sync if b == NB - 1 else None,
                )

        # software pipeline: P1(b+1) runs while block b's correction
        # streams land; corrections of b run while xh(b+2) lands.
        p1(0)
        for b in range(NB):
            if b + 1 < NB:
                if b + 2 < NB:
                    load_x(xh_blk, xh_pool, xh, f16, "xh", b + 2)
                load_x(xl_blk, xc_pool, xl, f8, "xl", b + 1)
                load_x(x8_blk, xc_pool, x8, f8, "x8", b + 1)
                p1(b + 1)
            corr(b)
    nc.compile()
    return nc


def _get_program(precision):
    key = f"nc_{precision}"
    if key not in _CACHE:
        _CACHE[key] = _build_hybrid8()
    return _CACHE[key]


def _xlayout(a, c):
    """[D, T_FULL] -> per-core [P, NB, KC, TB] (d = k*P + p, t = b*TB + tt)."""
    return np.ascontiguousarray(
        a.reshape(KC, P, NBF, TB)[:, :, c * NB:(c + 1) * NB, :].transpose(1, 2, 0, 3)
    )


def _wlayout(a):
    """[D, E] -> [P, KC, E]."""
    return np.ascontiguousarray(a.reshape(KC, P, E).transpose(1, 0, 2))


def kernel(x: np.ndarray, weight: np.ndarray, _trace: bool = False, **_kw):
    x = np.asarray(x, dtype=np.float32)
    weight = np.asarray(weight, dtype=np.float32)
    assert x.shape == (T_FULL, D) and weight.shape == (E, D)

    nc = _get_program(PRECISION)

    xt = np.ascontiguousarray(x.T)                       # [D, T_FULL]
    xh_full = xt.astype(np.float16)
    xl_full = ((xt - xh_full.astype(np.float32)) * np.float32(XL_SCALE)).astype(F8NP)
    x8_full = xh_full.astype(F8NP)

    wt = np.ascontiguousarray(weight.T)                  # [D, E]
    wt_s = wt * np.float32(S16)
    wh_flat = wt_s.astype(np.float16)
    wl_flat = (wt_s - wh_flat.astype(np.float32)).astype(F8NP)
    w8_flat = (wt * np.float32(W8_SCALE)).astype(F8NP)
    wh_h = _wlayout(wh_flat)
    w8_h = _wlayout(w8_flat)
    wl_h = _wlayout(wl_flat)

    in_maps = [
        {
            "xh": _xlayout(xh_full, c),
            "xl": _xlayout(xl_full, c),
            "x8": _xlayout(x8_full, c),
            "wh": wh_h,
            "w8": w8_h,
            "wl": wl_h,
        }
        for c in range(N_CORES)
    ]
    if _trace:
        import prof

        results, exec_time_ns, percore, neff_dir = prof.profiled_run(
            nc, in_maps, core_ids=list(range(N_CORES))
        )
        _CACHE["last_result"] = {
            "exec_time_ns": exec_time_ns,
            "percore": percore,
            "neff_dir": neff_dir,
        }
    else:
        res = run_bass_kernel_spmd(nc, in_maps, core_ids=list(range(N_CORES)))
        results = res.results
    w_full = np.concatenate([results[c]["w_out"] for c in range(N_CORES)], axis=0)
    i_full = np.concatenate(
        [results[c]["i_out"].astype(np.int32) for c in range(N_CORES)], axis=0
    )
    return w_full, i_full


# revision 10
# speedup vs baseline: 1.5713x; 1.0047x over previous
"""MoE group-limited routing gate (DeepSeek-style) on 8 Trainium2 NeuronCores.

Computation (per token t over E=256 experts, D=7168 features):
    logits = x @ weight.T                      [T, E]
    group-limited top-k: 8 groups of 32 experts, keep top-4 groups by
    group-max, then top-8 experts among kept groups.
    weights = sigmoid(logits[sel]) normalized to sum 1, * 2.5
Returns (weights [T,8] f32, indices [T,8] int32) like the reference.

Strategy: data-parallel over tokens, 2048 tokens/core, gate weight
replicated.  Matmul runs as one fp16 pass plus two fp8-e4m3 passes in
DoubleRow mode (2 k-chunks per instruction at 0.5 cycles/row), all
accumulating into a single PSUM tile via power-of-2 scale alignment:

    psum = xh @ (w<<16)_fp16  +  (xl<<11)_fp8 @ (w<<5)_fp8
                              +  (xh)_fp8 @ ((w - wh)<<16)_fp8
    logits = psum * 2^-16

fp16/fp8 products are exact in the f32 PSUM accumulator; the remaining
error is the fp8 rounding of the correction terms (~2e-5 absolute on
logits whose std is ~2.0), flipping ~7 of 131072 top-k slots.

Host pre-transposes into [128, block, chunk, token] layout so every DMA
lands 3.5-7KB contiguous runs per partition (vs 256-512B for a naive
[D, T] layout).  Top-k runs directly on the scaled PSUM (selection is
scale-invariant); the 2^-16 unscale rides the sigmoid activation's
scale input for free.
"""

import os
import numpy as np
from contextlib import ExitStack

import ml_dtypes

import concourse.bacc as bacc
import concourse.tile as tile
from concourse import mybir
from concourse.bass_utils import run_bass_kernel_spmd

N_CORES = 8
T_FULL = 16384
D = 7168
E = 256
G = 8            # expert groups
EPG = E // G     # experts per group = 32
TOPK = 8
TOPK_GROUPS = 4
ROUTE_SCALE = 2.5

P = 128
T = T_FULL // N_CORES       # 2048 tokens per core
KC = D // P                 # 56 contraction chunks
TB = 256                    # tokens per block
NB = T // TB                # 8 blocks per core
NBF = T_FULL // TB          # 64 blocks total
TPB = TB // P               # 2 token-tiles per block
KQ = 7                      # DMA splits per block / weight tensor
KCQ = KC // KQ              # 8 k-chunks per split (even: DoubleRow pairs)
NEG = -1.0e30
S16 = 2.0 ** 16             # scale of the PSUM accumulator
XL_SCALE = 2.0 ** 11        # xl fp8 pre-scale
W8_SCALE = 2.0 ** 5         # w8 fp8 pre-scale  (XL_SCALE*W8_SCALE == S16)
F8NP = ml_dtypes.float8_e4m3

PRECISION = os.environ.get("KPREC", "hybrid8")

_CACHE = {}


def _emit_topk(nc, sc_pool, out_pool, scores, wout, iout, t0, scale=1.0,
               out_eng=None):
    """Group-limited top-k + normalize on a [128, 256] f32 logits tile.

    scores may hold logits * (1/scale); selection is scale-invariant and
    the unscale is folded into the sigmoid activation."""
    f32 = mybir.dt.float32
    scores_g = scores.rearrange("p (g e) -> p g e", g=G)
    glog = sc_pool.tile([P, G], f32)
    nc.vector.reduce_max(out=glog, in_=scores_g, axis=mybir.AxisListType.X)
    gsort = sc_pool.tile([P, G], f32)
    nc.vector.max(out=gsort, in_=glog)
    # additive mask: 0 for kept groups (>= 4th-largest), -1e30 otherwise
    maskadd = sc_pool.tile([P, G], f32)
    nc.vector.tensor_scalar(
        out=maskadd,
        in0=glog,
        scalar1=gsort[:, TOPK_GROUPS - 1:TOPK_GROUPS],
        scalar2=NEG,
        op0=mybir.AluOpType.is_lt,
        op1=mybir.AluOpType.mult,
    )
    masked = sc_pool.tile([P, E], f32)
    nc.vector.tensor_add(
        masked.rearrange("p (g e) -> p g e", g=G),
        scores_g,
        maskadd.to_broadcast([P, G, EPG]),
    )
    top8 = sc_pool.tile([P, TOPK], f32)
    nc.vector.max(out=top8, in_=masked)
    idx = out_pool.tile([P, TOPK], mybir.dt.uint32)
    nc.vector.max_index(out=idx, in_max=top8, in_values=masked)
    sig = sc_pool.tile([P, TOPK], f32)
    nc.scalar.activation(
        out=sig, in_=top8, func=mybir.ActivationFunctionType.Sigmoid, scale=scale
    )
    ssum = sc_pool.tile([P, 1], f32)
    nc.vector.reduce_sum(out=ssum, in_=sig, axis=mybir.AxisListType.X)
    rec = sc_pool.tile([P, 1], f32)
    nc.vector.reciprocal(out=rec, in_=ssum)
    wres = out_pool.tile([P, TOPK], f32)
    nc.vector.tensor_scalar(
        out=wres,
        in0=sig,
        scalar1=rec[:, 0:1],
        scalar2=ROUTE_SCALE,
        op0=mybir.AluOpType.mult,
        op1=mybir.AluOpType.mult,
    )
    # outputs ride the SWDGE ring so the tiny writes never stall the
    # HWDGE ring that streams x; the last block uses the (by then idle)
    # sync HWDGE ring instead so the SWDGE drain overlaps compute.
    eng = out_eng if out_eng is not None else nc.gpsimd
    eng.dma_start(out=wout[t0:t0 + P, :], in_=wres)
    eng.dma_start(out=iout[t0:t0 + P, :], in_=idx)


def _build_hybrid8():
    nc = bacc.Bacc("TRN2", target_bir_lowering=False, debug=False, num_devices=N_CORES)
    f32 = mybir.dt.float32
    f16 = mybir.dt.float16
    f8 = mybir.dt.float8e4
    DR = mybir.MatmulPerfMode.DoubleRow

    xh = nc.dram_tensor("xh", [P, NB, KC, TB], f16, kind="ExternalInput").ap()
    xl = nc.dram_tensor("xl", [P, NB, KC, TB], f8, kind="ExternalInput").ap()
    x8 = nc.dram_tensor("x8", [P, NB, KC, TB], f8, kind="ExternalInput").ap()
    wh = nc.dram_tensor("wh", [P, KC, E], f16, kind="ExternalInput").ap()
    w8 = nc.dram_tensor("w8", [P, KC, E], f8, kind="ExternalInput").ap()
    wl = nc.dram_tensor("wl", [P, KC, E], f8, kind="ExternalInput").ap()
    wout = nc.dram_tensor("w_out", [T, TOPK], f32, kind="ExternalOutput").ap()
    iout = nc.dram_tensor("i_out", [T, TOPK], mybir.dt.uint32, kind="ExternalOutput").ap()

    with tile.TileContext(nc) as tc, ExitStack() as ctx:
        wt_pool = ctx.enter_context(tc.tile_pool(name="wt", bufs=1))
        xh_pool = ctx.enter_context(tc.tile_pool(name="xh", bufs=2))
        xc_pool = ctx.enter_context(tc.tile_pool(name="xc", bufs=2))
        psum_pool = ctx.enter_context(tc.tile_pool(name="psum", bufs=8, space="PSUM"))
        sc_pool = ctx.enter_context(tc.tile_pool(name="scratch", bufs=3))
        out_pool = ctx.enter_context(tc.tile_pool(name="outs", bufs=4))

        wh_sb, w8_sb, wl_sb = [], [], []

        # weights ride the Activation-engine HWDGE queue, in parallel with
        # the x streams on the sync queue: the prologue (weights + first two
        # x blocks) is DMA-bandwidth-bound, so two rings halve the ramp.
        def load_w(lst, src, dt_, name, q):
            t = wt_pool.tile([P, KCQ, E], dt_, tag=f"{name}{q}")
            nc.scalar.dma_start(out=t, in_=src[:, q * KCQ:(q + 1) * KCQ, :])
            lst.append(t)

        xh_blk, xl_blk, x8_blk = {}, {}, {}

        def load_x_split(dst, pool, src, dt_, name, b, q):
            t = pool.tile([P, KCQ, TB], dt_, tag=f"{name}{q}")
            nc.sync.dma_start(out=t, in_=src[:, b, q * KCQ:(q + 1) * KCQ, :])
            dst.setdefault(b, []).append(t)

        def load_x(dst, pool, src, dt_, name, b):
            for q in range(KQ):
                load_x_split(dst, pool, src, dt_, name, b, q)

        # DMA emission order == per-queue HWDGE consumption order.
        # Weight queue: wh, w8, wl.  x queue: xh(0), xh(1), xl(0), x8(0),
        # then steady-state per-block inside the loop.
        for q in range(KQ):
            load_w(wh_sb, wh, f16, "wh", q)
        for q in range(KQ):
            load_w(w8_sb, w8, f8, "w8", q)
        for q in range(KQ):
            load_w(wl_sb, wl, f8, "wl", q)
        load_x(xh_blk, xh_pool, xh, f16, "xh", 0)
        load_x(xh_blk, xh_pool, xh, f16, "xh", 1)
        load_x(xl_blk, xc_pool, xl, f8, "xl", 0)
        load_x(x8_blk, xc_pool, x8, f8, "x8", 0)

        psums = {}

        def p1(b):
            ps_list = []
            for j in range(TPB):
                js = slice(j * P, (j + 1) * P)
                ps = psum_pool.tile([P, E], f32)
                for k in range(KC):
                    nc.tensor.matmul(
                        ps,
                        xh_blk[b][k // KCQ][:, k % KCQ, js],
                        wh_sb[k // KCQ][:, k % KCQ, :],
                        start=(k == 0),
                        stop=False,
                    )
                ps_list.append(ps)
            psums[b] = ps_list

        def corr(b):
            for j in range(TPB):
                js = slice(j * P, (j + 1) * P)
                ps = psums[b][j]
                for q in range(KC // 2):
                    s = 2 * q
                    sp, so = s // KCQ, s % KCQ
                    nc.tensor.matmul(
                        ps,
                        xl_blk[b][sp][:, so:so + 2, js],
                        w8_sb[sp][:, so:so + 2, :],
                        start=False,
                        stop=False,
                        perf_mode=DR,
                    )
                for q in range(KC // 2):
                    s = 2 * q
                    sp, so = s // KCQ, s % KCQ
                    nc.tensor.matmul(
                        ps,
                        x8_blk[b][sp][:, so:so + 2, js],
                        wl_sb[sp][:, so:so + 2, :],
                        start=False,
                        stop=(q == KC // 2 - 1),
                        perf_mode=DR,
                    )
                _emit_topk(
                    nc, sc_pool, out_pool, ps, wout, iout, b * TB + j * P,
                    scale=1.0 / S16,
                    out_eng=nc.sync if b == NB - 1 else None,
                )

        # software pipeline: P1(b+1) runs while block b's correction
        # streams land; corrections of b run while xh(b+2) lands.
        p1(0)
        for b in range(NB):
            if b + 1 < NB:
                if b + 2 < NB:
                    load_x(xh_blk, xh_pool, xh, f16, "xh", b + 2)
                load_x(xl_blk, xc_pool, xl, f8, "xl", b + 1)
                load_x(x8_blk, xc_pool, x8, f8, "x8", b + 1)
                p1(b + 1)
            corr(b)
    nc.compile()
    return nc


def _get_program(precision):
    key = f"nc_{precision}"
    if key not in _CACHE:
        _CACHE[key] = _build_hybrid8()
    return _CACHE[key]


def _xlayout(a, c):
    """[D, T_FULL] -> per-core [P, NB, KC, TB] (d = k*P + p, t = b*TB + tt)."""
    return np.ascontiguousarray(
        a.reshape(KC, P, NBF, TB)[:, :, c * NB:(c + 1) * NB, :].transpose(1, 2, 0, 3)
    )


def _wlayout(a):
    """[D, E] -> [P, KC, E]."""
    return np.ascontiguousarray(a.reshape(KC, P, E).transpose(1, 0, 2))


def kernel(x: np.ndarray, weight: np.ndarray, _trace: bool = False, **_kw):
    x = np.asarray(x, dtype=np.float32)
    weight = np.asarray(weight, dtype=np.float32)
    assert x.shape == (T_FULL, D) and weight.shape == (E, D)

    nc = _get_program(PRECISION)

    xt = np.ascontiguousarray(x.T)                       # [D, T_FULL]
    xh_full = xt.astype(np.float16)
    xl_full = ((xt - xh_full.astype(np.float32)) * np.float32(XL_SCALE)).astype(F8NP)
    x8_full = xh_full.astype(F8NP)

    wt = np.ascontiguousarray(weight.T)                  # [D, E]
    wt_s = wt * np.float32(S16)
    wh_flat = wt_s.astype(np.float16)
    wl_flat = (wt_s - wh_flat.astype(np.float32)).astype(F8NP)
    w8_flat = (wt * np.float32(W8_SCALE)).astype(F8NP)
    wh_h = _wlayout(wh_flat)
    w8_h = _wlayout(w8_flat)
    wl_h = _wlayout(wl_flat)

    in_maps = [
        {
            "xh": _xlayout(xh_full, c),
            "xl": _xlayout(xl_full, c),
            "x8": _xlayout(x8_full, c),
            "wh": wh_h,
            "w8": w8_h,
            "wl": wl_h,
        }
        for c in range(N_CORES)
    ]
    if _trace:
        import prof

        results, exec_time_ns, percore, neff_dir = prof.profiled_run(
            nc, in_maps, core_ids=list(range(N_CORES))
        )
        _CACHE["last_result"] = {
            "exec_time_ns": exec_time_ns,
            "percore": percore,
            "neff_dir": neff_dir,
        }
    else:
        res = run_bass_kernel_spmd(nc, in_maps, core_ids=list(range(N_CORES)))
        results = res.results
    w_full = np.concatenate([results[c]["w_out"] for c in range(N_CORES)], axis=0)
    i_full = np.concatenate(
        [results[c]["i_out"].astype(np.int32) for c in range(N_CORES)], axis=0
    )
    return w_full, i_full
